# revision 1
# baseline (speedup 1.0000x reference)
"""Trainium2 Bass kernel for BiochemicalDynamics.

Reference computation (f32):
    Ax    = A @ x                                   # [N, DIM]
    s     = R * rowsum(x * Ax)                      # [N, 1]
    out   = F - B*x - s                             # [N, DIM]

Key identity used on-device: the output only needs the per-row scalar
    s_i = R * sum_j A[i,j] * <x_i, x_j> = R * rowsum_j (A ⊙ G)[i,j]
with G = x @ x.T. G tiles are produced on the TensorEngine from xT
(stationary xT[:, rows_i], moving xT[:, cols_j]) — so A is consumed in
its natural row-major layout and never needs a transpose. A single
fused VectorEngine op (tensor_tensor_reduce) multiplies the A chunk by
the G chunk from PSUM and row-reduces it, chaining the per-partition
accumulator across chunks.

Sharding: row-shard A (and x rows) across the 8 cores; every core gets
the full xT (the "all-gather of x" is done host-side by replicating the
2MB input). No cross-core reduction is needed.
"""

import sys

import numpy as np

for _p in ("/opt/trn_rl_repo", "/root/.axon_site/_ro/trn_rl_repo"):
    if _p not in sys.path:
        sys.path.append(_p)

N = 8192
DIM = 64
NCORES = 8
ROWS = N // NCORES  # 1024 rows of A per core

F_CONST = 1.0
B_CONST = 0.1
R_CONST = 0.01

P = 128                  # SBUF partitions
NSTRIPES = ROWS // P     # 8 row-stripes per core
CHUNK = 2048             # columns per fused multiply-reduce (4 PSUM banks)
NCHUNKS = N // CHUNK     # 4
MM_N = 512               # matmul moving free dim (one PSUM bank, f32)
MM_PER_CHUNK = CHUNK // MM_N

_CACHE = {}


def _build_nc():
    import concourse.mybir as mybir
    import concourse.tile as tile
    from concourse import bacc

    f32 = mybir.dt.float32
    f16 = mybir.dt.float16
    bf16 = mybir.dt.bfloat16

    nc = bacc.Bacc(
        trn_type="TRN2", target_bir_lowering=False, debug=False, num_devices=NCORES
    )

    # A is shipped as fp16 (host-side cast): uniform [0,1) values carry
    # <=2^-11 relative quantization error, which averages to ~4e-6 output
    # error over the 8192-term row reductions — while halving the HBM
    # traffic that dominates this memory-bound kernel.
    a = nc.dram_tensor("a", [ROWS, N], f16, kind="ExternalInput")
    # x^T split into bf16 (hi, lo) pairs: x = hi + lo to ~2^-17. The G
    # matmuls run in bf16 (4-5x faster than fp32 on PE) with f32 PSUM
    # accumulation. K=128 packing: the stationary stacks [hi; lo] along
    # the contraction axis (DIM=64 each half) and the moving tensors
    # carry hi (resp. lo) duplicated in both halves, so two K=128
    # matmuls accumulate the exact product (hi+lo)·(hi+lo)^T:
    # The stationary stacks [hi; lo] along K; the moving tensor carries hi
    # duplicated in both halves, so a single K=128 matmul per output slice
    # yields G ~= hi_l·hi_r + lo_l·hi_r. The dropped hi_l·lo_r term has
    # zero-mean random sign and averages out over the 8192x64 reduction
    # (~2e-6 relative) — far below the fp16-A quantization error.
    xlt_a = nc.dram_tensor("xlt_a", [2 * DIM, ROWS], bf16, kind="ExternalInput")
    xt2 = nc.dram_tensor("xt2", [2 * DIM, N], bf16, kind="ExternalInput")
    xloc = nc.dram_tensor("xloc", [ROWS, DIM], f32, kind="ExternalInput")
    out = nc.dram_tensor("out", [ROWS, DIM], f32, kind="ExternalOutput")

    mult = mybir.AluOpType.mult
    add = mybir.AluOpType.add

    with tile.TileContext(nc) as tc:
        with (
            tc.tile_pool(name="xpool", bufs=1) as xpool,
            tc.tile_pool(name="apool", bufs=6) as apool,
            tc.tile_pool(name="spool", bufs=2) as spool,
            tc.tile_pool(name="accpool", bufs=2 * NCHUNKS) as accpool,
            tc.tile_pool(name="psum", bufs=2, space="PSUM") as psum_pool,
        ):
            # One-time loads: stacked x^T operands for the G matmuls. The
            # stationaries and the first column-chunk of xt2 load first so
            # the first G matmuls (and the first A chunk's reduce) can
            # start while the rest of xt2 streams in.
            xlt_a_sb = xpool.tile([2 * DIM, ROWS], bf16)
            nc.sync.dma_start(out=xlt_a_sb[:], in_=xlt_a[:])
            # xt2 lands in pieces so the first (ramped) chunks' matmuls
            # wait on as little data as possible.
            xt2_sb = xpool.tile([2 * DIM, N], bf16)
            for o, w in ((0, MM_N), (MM_N, CHUNK - MM_N), (CHUNK, N - CHUNK)):
                nc.sync.dma_start(out=xt2_sb[:, o : o + w], in_=xt2[:, o : o + w])

            # Stripe 0 uses ramped chunk sizes so the very first reduce
            # only waits on 512 columns of A and x^T; later stripes use
            # full 2048-column chunks.
            RAMP = ((0, MM_N), (MM_N, CHUNK - MM_N),
                    (CHUNK, CHUNK), (2 * CHUNK, CHUNK), (3 * CHUNK, CHUNK))
            FULL = tuple((c * CHUNK, CHUNK) for c in range(NCHUNKS))
            for s in range(NSTRIPES):
                xl_sb = spool.tile([P, DIM], f32, tag="xl")
                nc.sync.dma_start(out=xl_sb[:], in_=xloc[s * P : (s + 1) * P, :])

                chunks = RAMP if s == 0 else FULL
                # acc4[:, c] = sum_j (A_chunk * R) * G_chunk  per chunk c,
                # via the fused DVE scalar_tensor_tensor accumulate output.
                acc4 = accpool.tile([P, len(RAMP)], f32, tag="acc4")
                lhsT_a = xlt_a_sb[:, s * P : (s + 1) * P]
                a_sb = apool.tile([P, N], f16, tag="a")
                for o, w in chunks if s == 0 else ((0, N),):
                    nc.sync.dma_start(
                        out=a_sb[:, o : o + w],
                        in_=a[s * P : (s + 1) * P, o : o + w],
                    )
                for ci, (o, w) in enumerate(chunks):
                    g_ps = psum_pool.tile([P, CHUNK], f32, tag="g")
                    for q in range(w // MM_N):
                        col = o + q * MM_N
                        nc.tensor.matmul(
                            g_ps[:, q * MM_N : (q + 1) * MM_N],
                            lhsT_a, xt2_sb[:, col : col + MM_N],
                            start=True, stop=True,
                        )
                    dummy = accpool.tile([P, 1], f32, tag="dummy")
                    nc.vector.scalar_tensor_tensor(
                        dummy.broadcast_to((P, w)),
                        a_sb[:, o : o + w],
                        R_CONST,
                        g_ps[:, :w],
                        op0=mult,
                        op1=mult,
                        accum_out=acc4[:, ci : ci + 1],
                    )

                # v = F - rowsum(acc4), fused into one idle-ScalarE op:
                # accum_out = sum_c(-acc4[:,c] + F/n) — keeps the reduce
                # off the VectorEngine, which is the kernel's bottleneck.
                vd = accpool.tile([P, len(RAMP)], f32, tag="vd")
                v = accpool.tile([P, 1], f32, tag="v")
                nc.scalar.activation(
                    vd[:, : len(chunks)], acc4[:, : len(chunks)],
                    mybir.ActivationFunctionType.Copy,
                    bias=F_CONST / len(chunks), scale=-1.0,
                    accum_out=v,
                )
                # out = Identity(x * -B + v) on ScalarE — back-to-back with
                # the v op above, keeping the epilogue off the VectorEngine.
                o_sb = spool.tile([P, DIM], f32, tag="o")
                nc.scalar.activation(
                    o_sb, xl_sb, mybir.ActivationFunctionType.Identity,
                    bias=v, scale=-B_CONST,
                )
                nc.sync.dma_start(out=out[s * P : (s + 1) * P, :], in_=o_sb[:])

    nc.finalize()
    return nc


def _get_nc():
    if "nc" not in _CACHE:
        _CACHE["nc"] = _build_nc()
    return _CACHE["nc"]


def _make_in_maps(x, A):
    import ml_dtypes

    bf16 = ml_dtypes.bfloat16
    x = np.ascontiguousarray(np.asarray(x, dtype=np.float32))
    A = np.ascontiguousarray(np.asarray(A, dtype=np.float32))
    xt = np.ascontiguousarray(x.T)
    xt_hi = xt.astype(bf16)
    xt_lo = (xt - xt_hi.astype(np.float32)).astype(bf16)
    xt2 = np.ascontiguousarray(np.vstack([xt_hi, xt_hi]))
    xlt_a = np.vstack([xt_hi, xt_lo])
    in_maps = []
    for c in range(NCORES):
        rows = slice(c * ROWS, (c + 1) * ROWS)
        in_maps.append(
            {
                "a": np.ascontiguousarray(A[rows]).astype(np.float16),
                "xt2": xt2,
                "xlt_a": np.ascontiguousarray(xlt_a[:, rows]),
                "xloc": np.ascontiguousarray(x[rows]),
            }
        )
    return in_maps


def run_sharded(x, A, trace=False, **kwargs):
    """Run the SPMD bass kernel; returns (full_output, BassKernelResults)."""
    from concourse.bass_utils import run_bass_kernel_spmd

    nc = _get_nc()
    res = run_bass_kernel_spmd(
        nc, _make_in_maps(x, A), core_ids=list(range(NCORES)), trace=trace, **kwargs
    )
    full = np.concatenate([res.results[c]["out"] for c in range(NCORES)], axis=0)
    return full.astype(np.float32, copy=False), res


def kernel(t, x, A):
    out, _ = run_sharded(x, A)
    return out



# revision 2
# speedup vs baseline: 1.9617x; 1.9617x over previous
"""Trainium2 Bass kernel for BiochemicalDynamics.

Reference computation (f32):
    Ax    = A @ x                                   # [N, DIM]
    s     = R * rowsum(x * Ax)                      # [N, 1]
    out   = F - B*x - s                             # [N, DIM]

Strategy (v2): compute Y^T = (A_local @ x)^T directly on the
TensorEngine by streaming A (host-side pre-transposed, fp8) as the
MOVING operand against stationary x row-chunks:

    Y^T[d, m] = sum_k x[k, d] * A[rows[m], k]
              = sum_kc matmul(lhsT = x[kc*128:(kc+1)*128, :],     # [K=128, M=64]
                              rhs  = A^T[kc*128:(kc+1)*128, m])   # [K=128, N<=512]

accumulated over all 64 K-chunks into one PSUM region [64, 1024].
fp8 DoubleRow packs two K-chunks per instruction (K=256), doubling the
PE column rate so the PE hides entirely under the A DMA stream.

The per-row dot s_i = R * <x_i, Y_i> then needs a PARTITION-axis
reduction of z = x^T .* Y^T, done with 8 tiny ones-vector matmuls
(lhsT = z[:, stripe], rhs = ones[64,1]) that land s in natural [128, 8]
layout for the ScalarE epilogue out = F - B*x - s.

A is quantized to fp8_e4m3 host-side: its rounding error is random-sign
and averages out over the 8192-term row reductions (measured 1.14e-3
max rel err vs the 2e-2 gate) while halving HBM traffic vs fp16 —
this kernel is DMA-bound on A (8MB/core at ~358 GB/s ~= 23us).

Sharding: row-shard A (1024 rows/core); every core gets the full x.
No cross-core communication.
"""

import sys

import numpy as np

for _p in ("/opt/trn_rl_repo", "/root/.axon_site/_ro/trn_rl_repo"):
    if _p not in sys.path:
        sys.path.append(_p)

N = 8192
DIM = 64
NCORES = 8
ROWS = N // NCORES       # 1024 rows of A per core
P = 128
NSTRIPES = ROWS // P     # 8 row-stripes per core
KC = N // P              # 64 contraction chunks of 128
KP = KC // 2             # 32 DoubleRow chunk-pairs
MH = ROWS // 512         # 2 moving halves of 512 output rows

F_CONST = 1.0
B_CONST = 0.1
R_CONST = 0.01

# A-chunk DMA schedule in kc units (pairs aligned): small chunks first so
# the first matmuls start early, then 1MB chunks for DMA efficiency.
A_CHUNKS = (2, 2, 4, 8, 8, 8, 8, 8, 8, 8)
assert sum(A_CHUNKS) == KC

_CACHE = {}


def _build_nc():
    import concourse.mybir as mybir
    import concourse.tile as tile
    from concourse import bacc

    f32 = mybir.dt.float32
    bf16 = mybir.dt.bfloat16
    f8 = mybir.dt.float8e4

    nc = bacc.Bacc(
        trn_type="TRN2", target_bir_lowering=False, debug=False, num_devices=NCORES
    )

    # at[p, kc, m] = A[rows_c[m], kc*128 + p]  (pre-transposed fp8 A)
    at = nc.dram_tensor("at", [P, KC, ROWS], f8, kind="ExternalInput")
    # xs[p, kc, d] = x[kc*128 + p, d]          (stationary chunks, fp8)
    xs = nc.dram_tensor("xs", [P, KC, DIM], f8, kind="ExternalInput")
    # xtd[d, m] = x[rows_c[m], d]              (f32, for the rowwise dot)
    xtd = nc.dram_tensor("xtd", [DIM, ROWS], f32, kind="ExternalInput")
    # xl[p, s*64+d] = x[rows_c[s*128+p], d]    (f32, for the epilogue)
    xl = nc.dram_tensor("xl", [P, NSTRIPES * DIM], f32, kind="ExternalInput")
    out = nc.dram_tensor("out", [P, NSTRIPES * DIM], f32, kind="ExternalOutput")

    mult = mybir.AluOpType.mult
    DR = mybir.MatmulPerfMode.DoubleRow

    with tile.TileContext(nc) as tc:
        with (
            tc.tile_pool(name="xpool", bufs=1) as xpool,
            tc.tile_pool(name="spool", bufs=1) as spool,
            tc.tile_pool(name="psum", bufs=1, space="PSUM") as psum_pool,
        ):
            # Small loads on the ACT HWDGE ring so they don't stall the
            # A stream on the SP ring.
            xs_sb = xpool.tile([P, KC, DIM], f8)
            nc.scalar.dma_start(out=xs_sb[:], in_=xs[:])
            xtd_sb = xpool.tile([DIM, ROWS], f32)
            nc.scalar.dma_start(out=xtd_sb[:], in_=xtd[:])
            xl_sb = xpool.tile([P, NSTRIPES * DIM], f32)
            nc.scalar.dma_start(out=xl_sb[:], in_=xl[:])
            ones_sb = xpool.tile([DIM, 1], bf16)
            nc.any.memset(ones_sb[:], 1.0)

            # A stream on the SP ring.
            at_sb = xpool.tile([P, KC, ROWS], f8)
            o = 0
            for w in A_CHUNKS:
                nc.sync.dma_start(out=at_sb[:, o : o + w, :], in_=at[:, o : o + w, :])
                o += w

            # Y^T accumulation: [64, 1024] f32 PSUM (2 banks).
            yt_ps = psum_pool.tile([DIM, ROWS], f32, tag="yt")
            for c in range(KP):
                lhsT = xs_sb[:, 2 * c : 2 * c + 2, :]
                for mh in range(MH):
                    nc.tensor.matmul(
                        yt_ps[:, mh * 512 : (mh + 1) * 512],
                        lhsT,
                        at_sb[:, 2 * c : 2 * c + 2, mh * 512 : (mh + 1) * 512],
                        start=(c == 0),
                        stop=(c == KP - 1),
                        perf_mode=DR,
                    )

            # z[d, m] = (xtd * R) * Y^T  -> bf16 (feeds the ones-matmuls)
            z_sb = spool.tile([DIM, ROWS], bf16, tag="z")
            nc.vector.scalar_tensor_tensor(
                z_sb[:], xtd_sb[:], R_CONST, yt_ps[:], op0=mult, op1=mult
            )

            # s[p, s] = sum_d z[d, s*128+p]  (partition reduce via matmul)
            s_ps = psum_pool.tile([P, NSTRIPES], f32, tag="s")
            for s in range(NSTRIPES):
                nc.tensor.matmul(
                    s_ps[:, s : s + 1],
                    z_sb[:, s * P : (s + 1) * P],
                    ones_sb[:],
                    start=True,
                    stop=True,
                )

            # v = F - s  (ScalarE), then out = Identity(xl * -B + v)
            v_sb = spool.tile([P, NSTRIPES], f32, tag="v")
            nc.scalar.activation(
                v_sb[:], s_ps[:],
                mybir.ActivationFunctionType.Copy,
                bias=F_CONST, scale=-1.0,
            )
            o_sb = spool.tile([P, NSTRIPES * DIM], f32, tag="o")
            for s in range(NSTRIPES):
                nc.scalar.activation(
                    o_sb[:, s * DIM : (s + 1) * DIM],
                    xl_sb[:, s * DIM : (s + 1) * DIM],
                    mybir.ActivationFunctionType.Identity,
                    bias=v_sb[:, s : s + 1],
                    scale=-B_CONST,
                )
            nc.sync.dma_start(out=out[:], in_=o_sb[:])

    nc.finalize()
    return nc


def _get_nc():
    if "nc" not in _CACHE:
        _CACHE["nc"] = _build_nc()
    return _CACHE["nc"]


def _make_in_maps(x, A):
    import ml_dtypes

    f8 = ml_dtypes.float8_e4m3
    x = np.ascontiguousarray(np.asarray(x, dtype=np.float32))
    A = np.asarray(A, dtype=np.float32)

    # xs[p, kc, d] = x[kc*128 + p, d]
    xs = np.ascontiguousarray(
        x.reshape(KC, P, DIM).transpose(1, 0, 2)
    ).astype(f8)

    in_maps = []
    for c in range(NCORES):
        rows = slice(c * ROWS, (c + 1) * ROWS)
        xc = x[rows]
        # at[p, kc, m] = A[rows[m], kc*128 + p]
        atq = A[rows].T.astype(f8)                       # [8192, 1024] fp8
        at = np.ascontiguousarray(
            atq.reshape(KC, P, ROWS).transpose(1, 0, 2)
        )
        in_maps.append(
            {
                "at": at,
                "xs": xs,
                "xtd": np.ascontiguousarray(xc.T),
                "xl": np.ascontiguousarray(
                    xc.reshape(NSTRIPES, P, DIM).transpose(1, 0, 2)
                ).reshape(P, NSTRIPES * DIM),
            }
        )
    return in_maps


def run_sharded(x, A, trace=False, **kwargs):
    """Run the SPMD bass kernel; returns (full_output, BassKernelResults)."""
    from concourse.bass_utils import run_bass_kernel_spmd

    nc = _get_nc()
    res = run_bass_kernel_spmd(
        nc, _make_in_maps(x, A), core_ids=list(range(NCORES)), trace=trace, **kwargs
    )
    full = np.concatenate(
        [
            res.results[c]["out"]
            .reshape(P, NSTRIPES, DIM)
            .transpose(1, 0, 2)
            .reshape(ROWS, DIM)
            for c in range(NCORES)
        ],
        axis=0,
    )
    return full.astype(np.float32, copy=False), res


def kernel(t, x, A):
    out, _ = run_sharded(x, A)
    return out


# revision 4
# speedup vs baseline: 2.1672x; 1.1048x over previous
"""Trainium2 Bass kernel for BiochemicalDynamics.

Reference computation (f32):
    Ax    = A @ x                                   # [N, DIM]
    s     = R * rowsum(x * Ax)                      # [N, 1]
    out   = F - B*x - s                             # [N, DIM]

Strategy: compute Y^T = (A_local @ x)^T directly on the TensorEngine by
streaming A (host-side pre-transposed, fp8) as the MOVING operand
against stationary x row-chunks:

    Y^T[d, m] = sum_kc matmul(lhsT = x[kc*128:(kc+1)*128, :],     # [K, M=64]
                              rhs  = A^T[kc*128:(kc+1)*128, m])   # [K, N]

accumulated over all 64 K-chunks into a PSUM region [64, 1024].
fp8 DoubleRow packs two K-chunks per instruction (K=256), giving the PE
enough column rate to hide entirely under the A DMA stream.

The per-row dot s_i = R * <x_i, Y_i> needs a PARTITION-axis reduction
of z = x^T .* Y^T, done with tiny ones-vector matmuls (lhsT =
z[:, stripe], rhs = ones[64,1]) that land s in natural [128, .] layout
for the ScalarE epilogue out = F - B*x - s.

A is streamed m-major in 4 quarters (256 output rows each): quarter q's
Y^T finishes while quarter q+1 is still streaming, so its reduction,
epilogue and output store all hide under the DMA stream — only the last
quarter's (short) chain sits in the tail.

A is quantized to fp8_e4m3 host-side: its rounding error is random-sign
and averages out over the 8192-term row reductions (measured ~1.6e-3
max rel err vs the 2e-2 gate) while halving HBM traffic vs fp16 —
this kernel is DMA-bound on A (8MB/core at ~341 GB/s ~= 23.5us).

Sharding: row-shard A (1024 rows/core); every core gets the full x.
No cross-core communication.
"""

import sys

import numpy as np

for _p in ("/opt/trn_rl_repo", "/root/.axon_site/_ro/trn_rl_repo"):
    if _p not in sys.path:
        sys.path.append(_p)

N = 8192
DIM = 64
NCORES = 8
ROWS = N // NCORES       # 1024 rows of A per core
P = 128
NSTRIPES = ROWS // P     # 8 row-stripes per core
KC = N // P              # 64 contraction chunks of 128
KP = KC // 2             # 32 DoubleRow chunk-pairs
NQ = 4                   # m-quarters (256 output rows each)
QW = ROWS // NQ          # 256

F_CONST = 1.0
B_CONST = 0.1
R_CONST = 0.01

# Per-quarter A DMA schedule in kc units. Quarter 0 ramps up so the
# first matmuls start early; later quarters use 1MB transfers.
Q_CHUNKS = (
    (2, 2, 4, 8, 16, 32),   # quarter 0 (ramp)
    (32, 32),
    (32, 32),
    (32, 32),
)
assert all(sum(ch) == KC for ch in Q_CHUNKS)

_CACHE = {}


def _build_nc():
    import concourse.mybir as mybir
    import concourse.tile as tile
    from concourse import bacc

    f32 = mybir.dt.float32
    bf16 = mybir.dt.bfloat16
    f8 = mybir.dt.float8e4

    nc = bacc.Bacc(
        trn_type="TRN2", target_bir_lowering=False, debug=False, num_devices=NCORES
    )

    # at[q, p, kc, j] = A[rows_c[q*256 + j], kc*128 + p]  (pre-transposed fp8 A)
    at = nc.dram_tensor("at", [NQ, P, KC, QW], f8, kind="ExternalInput")
    # xs[p, kc, d] = x[kc*128 + p, d]          (stationary chunks, fp8)
    xs = nc.dram_tensor("xs", [P, KC, DIM], f8, kind="ExternalInput")
    # xtd[d, m] = x[rows_c[m], d]              (f32, for the rowwise dot)
    xtd = nc.dram_tensor("xtd", [DIM, ROWS], f32, kind="ExternalInput")
    # xl[p, s*64+d] = x[rows_c[s*128+p], d]    (f32, for the epilogue)
    xl = nc.dram_tensor("xl", [P, NSTRIPES * DIM], f32, kind="ExternalInput")
    out = nc.dram_tensor("out", [P, NSTRIPES * DIM], f32, kind="ExternalOutput")

    mult = mybir.AluOpType.mult
    DR = mybir.MatmulPerfMode.DoubleRow

    with tile.TileContext(nc) as tc:
        with (
            tc.tile_pool(name="xpool", bufs=1) as xpool,
            tc.tile_pool(name="spool", bufs=1) as spool,
            tc.tile_pool(name="psum", bufs=1, space="PSUM") as psum_pool,
        ):
            # Small loads on the ACT HWDGE ring so they don't stall the
            # A stream on the SP ring. First xs piece is small so the
            # first matmul can start ASAP.
            xs_sb = xpool.tile([P, KC, DIM], f8)
            nc.scalar.dma_start(out=xs_sb[:, 0:8, :], in_=xs[:, 0:8, :])
            nc.scalar.dma_start(out=xs_sb[:, 8:, :], in_=xs[:, 8:, :])
            xtd_sb = xpool.tile([DIM, ROWS], f32)
            nc.scalar.dma_start(out=xtd_sb[:], in_=xtd[:])
            xl_sb = xpool.tile([P, NSTRIPES * DIM], f32)
            nc.scalar.dma_start(out=xl_sb[:], in_=xl[:])
            ones_sb = xpool.tile([DIM, 1], bf16)
            nc.any.memset(ones_sb[:], 1.0)

            # A stream on the SP ring, quarter-major.
            at_sb = [
                xpool.tile([P, KC, QW], f8, name=f"at_sb{q}", tag=f"at{q}")
                for q in range(NQ)
            ]
            for q in range(NQ):
                o = 0
                for w in Q_CHUNKS[q]:
                    nc.sync.dma_start(
                        out=at_sb[q][:, o : o + w, :], in_=at[q, :, o : o + w, :]
                    )
                    o += w

            # Y^T accumulation: [64, 1024] f32 PSUM (2 banks).
            yt_ps = psum_pool.tile([DIM, ROWS], f32, tag="yt")
            z_sb = spool.tile([DIM, ROWS], bf16, tag="z")
            s_ps = psum_pool.tile([P, NSTRIPES], f32, tag="s")
            v_sb = spool.tile([P, NSTRIPES], f32, tag="v")
            o_sb = spool.tile([P, NSTRIPES * DIM], f32, tag="o")

            def pe_reduce(q):
                # s[p, 2q+i] = sum_d z[d, (2q+i)*128 + p]
                for s in (2 * q, 2 * q + 1):
                    nc.tensor.matmul(
                        s_ps[:, s : s + 1],
                        z_sb[:, s * P : (s + 1) * P],
                        ones_sb[:],
                        start=True,
                        stop=True,
                    )

            def scalar_epilogue(q):
                # v = F - s, then out = Identity(xl * -B + v), store.
                nc.scalar.activation(
                    v_sb[:, 2 * q : 2 * q + 2], s_ps[:, 2 * q : 2 * q + 2],
                    mybir.ActivationFunctionType.Copy,
                    bias=F_CONST, scale=-1.0,
                )
                for s in (2 * q, 2 * q + 1):
                    nc.scalar.activation(
                        o_sb[:, s * DIM : (s + 1) * DIM],
                        xl_sb[:, s * DIM : (s + 1) * DIM],
                        mybir.ActivationFunctionType.Identity,
                        bias=v_sb[:, s : s + 1],
                        scale=-B_CONST,
                    )
                nc.scalar.dma_start(
                    out=out[:, q * 2 * DIM : (q + 1) * 2 * DIM],
                    in_=o_sb[:, q * 2 * DIM : (q + 1) * 2 * DIM],
                )

            for q in range(NQ):
                for c in range(KP):
                    nc.tensor.matmul(
                        yt_ps[:, q * QW : (q + 1) * QW],
                        xs_sb[:, 2 * c : 2 * c + 2, :],
                        at_sb[q][:, 2 * c : 2 * c + 2, :],
                        start=(c == 0),
                        stop=(c == KP - 1),
                        perf_mode=DR,
                    )
                # z[d, m] = (xtd * R) * Y^T  for this quarter -> bf16
                nc.vector.scalar_tensor_tensor(
                    z_sb[:, q * QW : (q + 1) * QW],
                    xtd_sb[:, q * QW : (q + 1) * QW],
                    R_CONST,
                    yt_ps[:, q * QW : (q + 1) * QW],
                    op0=mult,
                    op1=mult,
                )
                # Emit quarter q-1's PE reduction AFTER quarter q's matmuls
                # so the PE never stalls waiting for the DVE mid-stream.
                if q > 0:
                    pe_reduce(q - 1)
                    scalar_epilogue(q - 1)
            pe_reduce(NQ - 1)
            scalar_epilogue(NQ - 1)

    nc.finalize()
    return nc


def _get_nc():
    if "nc" not in _CACHE:
        _CACHE["nc"] = _build_nc()
    return _CACHE["nc"]


def _make_in_maps(x, A):
    import ml_dtypes

    f8 = ml_dtypes.float8_e4m3
    x = np.ascontiguousarray(np.asarray(x, dtype=np.float32))
    A = np.asarray(A, dtype=np.float32)

    # xs[p, kc, d] = x[kc*128 + p, d]
    xs = np.ascontiguousarray(
        x.reshape(KC, P, DIM).transpose(1, 0, 2)
    ).astype(f8)

    in_maps = []
    for c in range(NCORES):
        rows = slice(c * ROWS, (c + 1) * ROWS)
        xc = x[rows]
        # at[q, p, kc, j] = A[rows[q*256 + j], kc*128 + p]
        atq = A[rows].T.astype(f8)                       # [8192, 1024] fp8
        at = np.ascontiguousarray(
            atq.reshape(KC, P, NQ, QW).transpose(2, 1, 0, 3)
        )
        in_maps.append(
            {
                "at": at,
                "xs": xs,
                "xtd": np.ascontiguousarray(xc.T),
                "xl": np.ascontiguousarray(
                    xc.reshape(NSTRIPES, P, DIM).transpose(1, 0, 2)
                ).reshape(P, NSTRIPES * DIM),
            }
        )
    return in_maps


def run_sharded(x, A, trace=False, **kwargs):
    """Run the SPMD bass kernel; returns (full_output, BassKernelResults)."""
    from concourse.bass_utils import run_bass_kernel_spmd

    nc = _get_nc()
    res = run_bass_kernel_spmd(
        nc, _make_in_maps(x, A), core_ids=list(range(NCORES)), trace=trace, **kwargs
    )
    full = np.concatenate(
        [
            res.results[c]["out"]
            .reshape(P, NSTRIPES, DIM)
            .transpose(1, 0, 2)
            .reshape(ROWS, DIM)
            for c in range(NCORES)
        ],
        axis=0,
    )
    return full.astype(np.float32, copy=False), res


def kernel(t, x, A):
    out, _ = run_sharded(x, A)
    return out
